# revision 72
# baseline (speedup 1.0000x reference)
"""Bahdanau-style attention kernel for Trainium2, SPMD across 8 NeuronCores.

Math (per batch row b):
    dec_proj = decoder_state @ W_dec + b_transform            # [D]
    enc_proj = encoder_outputs[b] @ W_enc                     # [S, D]
    feats    = tanh(enc_proj + dec_proj)                      # [S, D]
    scores   = feats @ v_scorer                               # [S]
    probs    = softmax(where(mask, scores, -1e9))             # [S]

Distribution: data-parallel on batch (weights replicated) with mask-aware
work packing. The mask is length-style, so positions >= length contribute
exactly 0. Work is split into variable-width units (one contiguous chunk
of one batch, width <= 512); a COMMON width template T[0..U-1] is chosen
so every core's sorted chunk profile fits under it (SPMD: all cores run
the same graph). Template slots are first-fit packed into "bins" of
<= 512 columns; each bin is one PSUM bank and one matmul per (t, e)
weight tile, so the 512-column moving operand amortizes LDWEIGHTS and
per-instruction overheads.

Device-side formulation (per core):
  - Transposed layout enc_projT[d, s]: lhsT (stationary) = W_enc tiles in
    natural [e, d] layout, moving operand = encT[e, s] (host pre-packs
    bins contiguously per partition + casts bf16), dec_proj bias rides
    the partition axis (native ScalarE activation bias), v-dot is a
    K=D matmul per unit.
  - Unit u's scores land on partition u of one PSUM tile [U, 512] via
    one-hot-column v selectors; the additive log-mask matmul OPENS the
    accumulation group covering the full [U, 512] (variable-width score
    matmuls only cover prefixes), score matmuls accumulate behind it.
  - Softmax without max-subtraction (scores are tanh-bounded):
    probs = exp(s + logmask) / sum; per-batch sums are formed from
    per-unit row sums with two tiny matmuls against 0/1 unit<->slot
    maps, so no cross-partition vector ops are needed.
"""

import math
import random

import numpy as np
import ml_dtypes

# NOTE: walrus's --enable-ldw-opt pass (which could elide redundant
# LDWEIGHTS for same-weight matmul runs) rejects bass's pre-split
# InstLdweights stream, so DoubleRow matmuls stay weight-load-bound
# (~310ns vs ~146ns stream time) — no way to amortize reloads from here.

B, S, E, D = 64, 1024, 1024, 512
N_CORES = 8
BPC = B // N_CORES  # batch slots per core
NE = E // 128  # 8 e-tiles
ND = D // 128  # 4 d-tiles
NK = D // 128  # 4 k-tiles for W_dec (K = DEC_DIM = 512)
MAXW = 512  # PSUM bank width (512 f32) — scU/out shapes
# max unit/bin width: matmuls at exactly N=512 (full PSUM bank) measure
# ~+54ns slower than N<=508, and fp8 DoubleRow subtile steps must be
# 16-byte aligned, so widths are 16-aligned and capped at 496
CAPW = 496
ALIGN = 16
# e-tile pairs computed in fp8e4m3 via DoubleRow (out of NE//2 = 4 pairs);
# remaining e-tiles stay bf16.  2 pairs (half the contraction) measures
# rel_err ~1.5e-2 in simulation vs the 2e-2 gate; 0 = all-bf16 fallback.
FP8_PAIRS = 2
WSCALE = 32.0  # pow-2 prescale on W_enc so fp8 weights avoid denormals
NEG = -1e9

_cache = {}


# ---------------------------------------------------------------------------
# packing: choose per-core batch assignment + common width template
# ---------------------------------------------------------------------------

def _chunks(l):
    """Split length l into ceil(l/CAPW) near-equal pieces (desc order)."""
    k = max(1, math.ceil(l / CAPW))
    base, rem = divmod(l, k)
    return sorted(
        (base + (1 if i < rem else 0) for i in range(k)), reverse=True
    )


def _profile(core_batches, lengths, U):
    ws = []
    for b in core_batches:
        ws += _chunks(int(lengths[b]))
    ws.sort(reverse=True)
    if len(ws) > U:
        return None
    return ws + [0] * (U - len(ws))


def _template_cost(members, lengths, U):
    profs = [_profile(m, lengths, U) for m in members]
    if any(p is None for p in profs):
        return None, None
    T = [
        min(
            CAPW,
            (max(p[j] for p in profs) + ALIGN - 1) // ALIGN * ALIGN,
        )
        for j in range(U)
    ]
    T = [t for t in T if t > 0]
    return sum(T), T


def _assign(lengths):
    """Assign batches to cores + build the common width template.

    Returns (members, T): members[i] = batch ids on core i (slot order),
    T = template widths (desc, 4-aligned, <= MAXW).
    """
    order = sorted(range(B), key=lambda b: -int(lengths[b]))
    loads = [0] * N_CORES
    members = [[] for _ in range(N_CORES)]
    for b in order:
        cands = [i for i in range(N_CORES) if len(members[i]) < BPC]
        i = min(cands, key=lambda i: (loads[i], len(members[i])))
        members[i].append(b)
        loads[i] += int(lengths[b])

    U = max(
        sum(math.ceil(max(1, int(lengths[b])) / MAXW) for b in m)
        for m in members
    ) + 1
    rng = random.Random(1234)
    bestc, bestT = _template_cost(members, lengths, U)
    bestm = [list(m) for m in members]
    cur = [list(m) for m in members]
    curc = bestc
    for _ in range(30000):
        i = rng.randrange(N_CORES)
        j = rng.randrange(N_CORES)
        if i == j:
            continue
        a = rng.randrange(len(cur[i]))
        b2 = rng.randrange(len(cur[j]))
        cur[i][a], cur[j][b2] = cur[j][b2], cur[i][a]
        c, T = _template_cost(cur, lengths, U)
        if c is not None and c <= curc:
            curc = c
            if c < bestc:
                bestc, bestT = c, T
                bestm = [list(m) for m in cur]
        else:
            cur[i][a], cur[j][b2] = cur[j][b2], cur[i][a]
    return bestm, bestT


def _bins_of(T):
    """First-fit-decreasing template slots into bins of <= CAPW columns.

    Returns list of bins; each bin is a list of slot indices.
    """
    bins = []
    widths = []
    for j in range(len(T)):  # T already sorted desc
        for bi, w in enumerate(widths):
            if w + T[j] <= CAPW:
                bins[bi].append(j)
                widths[bi] += T[j]
                break
        else:
            bins.append([j])
            widths.append(T[j])
    # NOTE: sorting multi-slot bins first (to shrink the tail's last-ACT
    # wait) measured ~1us WORSE — heavier early ACT load during the
    # DMA-tight startup outweighs the tail gain; keep FFD order
    return bins


# ---------------------------------------------------------------------------
# device graph
# ---------------------------------------------------------------------------

def _build(T):
    """Build + compile the SPMD graph for width template T."""
    from contextlib import ExitStack

    import concourse.bass as bass
    import concourse.tile as tile
    from concourse import bacc, mybir

    f32 = mybir.dt.float32
    bf16 = mybir.dt.bfloat16
    f8 = mybir.dt.float8e4
    AF = mybir.ActivationFunctionType
    DR = mybir.MatmulPerfMode.DoubleRow

    U = len(T)
    bins = _bins_of(T)
    NG = len(bins)
    binw = [sum(T[j] for j in bb) for bb in bins]  # bin total width
    # per-slot offset within its bin
    sloff = {}
    for bb in bins:
        off = 0
        for j in bb:
            sloff[j] = off
            off += T[j]
    NF8 = 2 * FP8_PAIRS  # e-tiles in fp8
    NBF = NE - NF8  # e-tiles in bf16
    F8W = NF8 * MAXW  # per-partition flat capacity of a bin's fp8 blob
    FBW = NBF * MAXW  # ... and its bf16 blob

    nc = bacc.Bacc(
        "TRN2", target_bir_lowering=False, debug=False, num_devices=N_CORES
    )

    if NF8:
        enc8 = nc.dram_tensor("enc8", [NG, 128, F8W], f8, kind="ExternalInput").ap()
        wenc8 = nc.dram_tensor(
            "wenc8", [128, ND, FP8_PAIRS, 2, 128], f8, kind="ExternalInput"
        ).ap()
    encU = nc.dram_tensor("encU", [NG, 128, FBW], bf16, kind="ExternalInput").ap()
    wenc = nc.dram_tensor("wenc", [ND, 128, NBF, 128], bf16, kind="ExternalInput").ap()
    wdec = nc.dram_tensor("wdec", [128, NK, D], bf16, kind="ExternalInput").ap()
    dectU = nc.dram_tensor("dectU", [128, NK, U], bf16, kind="ExternalInput").ap()
    bt = nc.dram_tensor("bt", [128, ND], f32, kind="ExternalInput").ap()
    # multi-hot v selector: vGT[:, g, t, j] = v tile t if slot j is in bin g
    # else 0 — ONE score matmul per (bin, t) scores every unit of the bin
    # simultaneously; stray products land on masked columns of other rows
    vGT = nc.dram_tensor("vGT", [128, NG, ND, U], bf16, kind="ExternalInput").ap()
    # additive log-mask (0 valid / -1e9 masked) + identity; its matmul OPENS
    # the score accumulation group (start=True) covering the full [U, MAXW]
    maskl = nc.dram_tensor("maskl", [U, MAXW], bf16, kind="ExternalInput").ap()
    idU = nc.dram_tensor("idU", [U, U], bf16, kind="ExternalInput").ap()
    # unit<->unit same-batch indicator: M[u', u] = 1 iff batch(u') == batch(u);
    # one matmul turns per-unit row sums into per-unit BATCH sums directly
    u2u = nc.dram_tensor("u2u", [U, U], f32, kind="ExternalInput").ap()
    out = nc.dram_tensor("out", [U, MAXW], f32, kind="ExternalOutput").ap()

    with tile.TileContext(nc) as tc:
        with ExitStack() as ctx:
            const = ctx.enter_context(tc.tile_pool(name="const", bufs=1))
            enc_pool = ctx.enter_context(tc.tile_pool(name="encp", bufs=6))
            fpool = ctx.enter_context(tc.tile_pool(name="feats", bufs=12))

            # PE warmup: the memset is the very first gpsimd instruction so
            # the dependency-free dummy matmuls can start ASAP, fill the
            # startup DMA wait, and trip the HAM clock-gate up.  Sized to
            # END as bin 0's data lands (~2us): 12 N=512 warmups measured
            # ~5.2us at cold clock and DELAYED the first real matmul ~3.3us
            warm_sb = const.tile([128, 384], bf16)
            nc.gpsimd.memset(warm_sb[:], 0.0)

            # HBM bandwidth is the startup constraint, so everything rides
            # ONE ring in dependency order: dec-phase weights, then the t=0
            # d-tile of W_enc + bin 0 (split per-e so the first matmul only
            # waits for its own slice), then the rest
            wdec_sb = const.tile([128, NK, D], bf16)
            nc.sync.dma_start(wdec_sb[:], wdec)
            dectU_sb = const.tile([128, NK, U], bf16)
            nc.sync.dma_start(dectU_sb[:], dectU)
            # tiny, but consumed by the score-group opener matmul which sits
            # early in the PE program order — load before the big bin DMAs
            maskl_sb = const.tile([U, MAXW], bf16)
            nc.sync.dma_start(maskl_sb[:], maskl)
            idU_sb = const.tile([U, U], bf16)
            nc.sync.dma_start(idU_sb[:], idU)
            wenc_sb = const.tile([128, ND, NBF, 128], bf16)
            if NF8:
                w8_sb = const.tile([128, ND, FP8_PAIRS, 2, 128], f8)
                nc.sync.dma_start(w8_sb[:], wenc8)

            def load_bin(g, split=False):
                W = binw[g]
                et8 = None
                if NF8:
                    et8 = enc_pool.tile([128, F8W], f8, tag="et8", name="et8")
                    if split:
                        for k in range(NF8):
                            nc.sync.dma_start(
                                et8[:, k * W : (k + 1) * W],
                                enc8[g, :, k * W : (k + 1) * W],
                            )
                    else:
                        nc.sync.dma_start(
                            et8[:, 0 : NF8 * W], enc8[g, :, 0 : NF8 * W]
                        )
                etb = enc_pool.tile([128, FBW], bf16, tag="etb", name="etb")
                if split:
                    for e in range(NBF):
                        nc.sync.dma_start(
                            etb[:, e * W : (e + 1) * W],
                            encU[g, :, e * W : (e + 1) * W],
                        )
                else:
                    nc.sync.dma_start(etb[:, 0 : NBF * W], encU[g, :, 0 : NBF * W])
                return et8, etb

            ets = []
            npre = min(2, NG)
            nc.sync.dma_start(wenc_sb[:, 0, :, :], wenc[0])
            ets.append(load_bin(0, split=True))
            for t in range(1, ND):
                nc.sync.dma_start(wenc_sb[:, t, :, :], wenc[t])
            for g in range(1, npre):
                # NOTE: splitting later bins per-piece (like bin 0) measures
                # 14us WORSE — 8x more, smaller DMA descriptors slow the
                # whole ring; whole-blob loads keep descriptors at ~4-6KB
                ets.append(load_bin(g))
            bt_sb = const.tile([128, ND], f32)
            nc.scalar.dma_start(bt_sb[:], bt)
            # late-needed constants go on the sync ring BEHIND the bin
            # prefetches so they don't steal HBM bandwidth at startup
            vGT_sb = const.tile([128, NG, ND, U], bf16)
            nc.sync.dma_start(vGT_sb[:], vGT)
            u2u_sb = const.tile([U, U], f32)
            nc.sync.dma_start(u2u_sb[:], u2u)
            decprojU_sb = const.tile([128, ND, U], f32)

            # 3 dummy warmups trip the clock-gate; the dec phase right after
            # is REAL work that continues warming the PE while bin 0's data
            # is still in flight, so bin 0 starts as soon as it lands
            with tc.tile_pool(name="warmp", bufs=1, space="PSUM") as wpool:
                wps = wpool.tile([128, 256], f32, name="wps")
                for _ in range(3):
                    nc.tensor.matmul(
                        wps[:],
                        lhsT=warm_sb[:, 0:128],
                        rhs=warm_sb[:, 128:384],
                        start=True,
                        stop=True,
                        skip_group_check=True,
                    )

            # --- dec_projT[d, u] = W_dec^T @ dec_colsU + b ---
            with tc.tile_pool(name="dpsum", bufs=2, space="PSUM") as dpsum:
                for t in range(ND):
                    ps = dpsum.tile([128, U], f32, name="dps")
                    for k in range(NK):
                        nc.tensor.matmul(
                            ps[:],
                            lhsT=wdec_sb[:, k, bass.ts(t, 128)],
                            rhs=dectU_sb[:, k, :],
                            start=(k == 0),
                            stop=(k == NK - 1),
                        )
                    nc.scalar.add(decprojU_sb[:, t, :], ps[:], bt_sb[:, t : t + 1])

            spsum = ctx.enter_context(tc.tile_pool(name="spsum", bufs=1, space="PSUM"))
            scU = spsum.tile([U, MAXW], f32, name="scU")
            n_sc_mms = NG * ND + 1  # +1 for the log-mask opener
            state = {"count": 0}
            pending = []  # delayed score MMs: (u, t, ft)

            # the log-mask matmul opens the accumulation group, covering the
            # full [U, MAXW] so the variable-width score matmuls (which only
            # write prefixes) always accumulate onto initialized PSUM
            state["count"] += 1
            nc.tensor.matmul(
                scU[:],
                lhsT=idU_sb[:],
                rhs=maskl_sb[:],
                start=True,
                stop=False,
                skip_group_check=True,
            )

            def flush_pending():
                for (g2, t, ft) in pending:
                    w = binw[g2]
                    state["count"] += 1
                    nc.tensor.matmul(
                        scU[:, 0:w],
                        lhsT=vGT_sb[:, g2, t, :],
                        rhs=ft[:, 0:w],
                        start=False,
                        stop=(state["count"] == n_sc_mms),
                        skip_group_check=True,
                    )
                pending.clear()

            # --- main loop over bins ---
            # 7 main-loop banks + the score bank = all 8 PSUM banks; the
            # extra sweep in flight buys slack against tanh-ACT drain
            with tc.tile_pool(name="mpsum", bufs=7, space="PSUM") as mpsum:
                for g in range(NG):
                    W = binw[g]
                    if g < npre:
                        et8, etb = ets[g]
                    else:
                        et8, etb = load_bin(g)
                    prev = list(pending)
                    pending.clear()
                    this_bin = []
                    for t in range(ND):
                        ps = mpsum.tile([128, MAXW], f32, tag="mp", name="mp")
                        for p in range(FP8_PAIRS):
                            # DoubleRow: lhsT [128, 2, 128] / rhs [128, 2, W]
                            # contract BOTH e-subtiles of the pair in one MM
                            nc.tensor.matmul(
                                ps[:, 0:W],
                                lhsT=w8_sb[:, t, p, :, :],
                                rhs=et8[:, 2 * p * W : (2 * p + 2) * W].rearrange(
                                    "a (k w) -> a k w", k=2
                                ),
                                start=(p == 0),
                                stop=False,
                                perf_mode=DR,
                            )
                        for e in range(NBF):
                            nc.tensor.matmul(
                                ps[:, 0:W],
                                lhsT=wenc_sb[:, t, e, :],
                                rhs=etb[:, e * W : e * W + W],
                                start=(NF8 == 0 and e == 0),
                                stop=(e == NBF - 1),
                            )
                        if t == 0 and prev:
                            # emit previous bin's score MMs behind the first
                            # weight sweep (their tanh inputs are ready, so
                            # PE doesn't stall on ACT)
                            pending.extend(prev)
                            prev = []
                            flush_pending()
                        # one shared feature tile per (bin, t); ACTs write
                        # disjoint slices, the multi-hot score MM reads it all
                        ft = fpool.tile([128, MAXW], bf16, tag="ft", name="ft")
                        for j in bins[g]:
                            off = sloff[j]
                            nc.scalar.activation(
                                ft[:, off : off + T[j]],
                                ps[:, off : off + T[j]],
                                func=AF.Tanh,
                                bias=decprojU_sb[:, t, j : j + 1],
                                scale=1.0 / WSCALE,
                            )
                        this_bin.append((g, t, ft))
                    pending.extend(this_bin)
                flush_pending()

            # --- masked softmax epilogue in unit space ---
            with tc.tile_pool(name="tpsum", bufs=1, space="PSUM") as tpsum, \
                 tc.tile_pool(name="epi", bufs=1) as epool:
                escU = epool.tile([U, MAXW], f32, name="escU")
                usums = epool.tile([U, 1], f32, name="usums")
                nc.scalar.activation(
                    escU[:], scU[:], func=AF.Exp, accum_out=usums[:]
                )
                # per-unit BATCH sums in one matmul: bU[u] = sum_u' M[u',u]*usums[u']
                bU_ps = tpsum.tile([U, 1], f32, name="bU_ps")
                nc.tensor.matmul(
                    bU_ps[:], lhsT=u2u_sb[:], rhs=usums[:], start=True, stop=True
                )
                rbU = epool.tile([U, 1], f32, name="rbU")
                nc.vector.reciprocal(rbU[:], bU_ps[:])
                # split the final scale + store into column halves on two
                # DMA rings so the first store overlaps the second scale
                probsU = epool.tile([U, MAXW], f32, name="probsU")
                H = MAXW // 2
                nc.vector.tensor_scalar_mul(probsU[:, 0:H], escU[:, 0:H], rbU[:])
                nc.sync.dma_start(out[:, 0:H], probsU[:, 0:H])
                nc.vector.tensor_scalar_mul(probsU[:, H:MAXW], escU[:, H:MAXW], rbU[:])
                nc.scalar.dma_start(out[:, H:MAXW], probsU[:, H:MAXW])

    nc.compile()
    return nc


# ---------------------------------------------------------------------------
# host-side data prep
# ---------------------------------------------------------------------------

def _prep_inputs(decoder_state, encoder_outputs, input_mask, W_transform,
                 b_transform, v_scorer, members, T):
    bf16 = ml_dtypes.bfloat16
    f8 = ml_dtypes.float8_e4m3
    W_dec = W_transform[:D]
    W_enc = W_transform[D:]
    U = len(T)
    bins = _bins_of(T)
    NG = len(bins)
    binw = [sum(T[j] for j in bb) for bb in bins]
    sloff = {}
    for bb in bins:
        off = 0
        for j in bb:
            sloff[j] = off
            off += T[j]
    NF8 = 2 * FP8_PAIRS
    NBF = NE - NF8
    F8W = NF8 * MAXW
    FBW = NBF * MAXW

    W32 = W_enc * np.float32(WSCALE)
    # bf16 e-tiles NF8..NE-1: [e, d] -> [t_d, p, t_e, dcol]
    wenc_h = np.ascontiguousarray(
        W32.astype(bf16).reshape(NE, 128, ND, 128).transpose(2, 1, 0, 3)[
            :, :, NF8:, :
        ]
    )
    if NF8:
        # fp8 e-tiles as DoubleRow pairs: [128, ND, pair, sub, dcol]
        wenc8_h = np.ascontiguousarray(
            W32[: NF8 * 128]
            .astype(f8)
            .reshape(FP8_PAIRS, 2, 128, ND, 128)
            .transpose(2, 3, 0, 1, 4)
        )
    wdec_h = np.ascontiguousarray(
        W_dec.astype(bf16).reshape(NK, 128, D).transpose(1, 0, 2)
    )
    bt_h = np.ascontiguousarray(b_transform.astype(np.float32).reshape(ND, 128).T)
    v_tiles = v_scorer.astype(np.float32).reshape(ND, 128).T.astype(bf16)  # [128, ND]

    lengths = input_mask.sum(axis=1).astype(int)
    enc_bf = encoder_outputs.astype(bf16)  # [B, S, E]
    enc_f8 = encoder_outputs.astype(f8) if NF8 else None
    dec_bf = decoder_state.astype(bf16)  # [B, D]

    in_maps = []
    unit_maps = []  # per core: list of (global_batch, s0, n) per slot (None = dummy)
    for core in range(N_CORES):
        batches = members[core]
        # per-core units: even chunks, sorted desc — matches template order
        units = []  # (width, gb, slot_in_core, s0)
        for slot, gb in enumerate(batches):
            l = int(lengths[gb])
            s0 = 0
            for wdt in _chunks(l):
                units.append((wdt, gb, slot, s0))
                s0 += wdt
        units.sort(key=lambda x: -x[0])
        assert len(units) <= U
        assert all(u[0] <= T[j] for j, u in enumerate(units))

        encU_h = np.zeros((NG, 128, FBW), dtype=bf16)
        enc8_h = np.zeros((NG, 128, F8W), dtype=f8) if NF8 else None
        dectU_h = np.zeros((128, NK, U), dtype=bf16)
        vGT_h = np.zeros((128, NG, ND, U), dtype=bf16)
        for bi, bb in enumerate(bins):
            for j in bb:
                for t in range(ND):
                    vGT_h[:, bi, t, j] = v_tiles[:, t]
        maskl_h = np.full((U, MAXW), NEG, dtype=bf16)
        idU_h = np.eye(U, dtype=bf16)
        u2s_h = np.zeros((U, BPC), dtype=np.float32)
        umap = [None] * U
        for j, (n, gb, slot, s0) in enumerate(units):
            g = next(bi for bi, bb in enumerate(bins) if j in bb)
            W = binw[g]
            off = sloff[j]
            # [n, E] -> [E, n] -> [NE, 128, n] -> [128, NE, n]
            blk = enc_bf[gb, s0 : s0 + n]
            bT = blk.T.reshape(NE, 128, n).transpose(1, 0, 2)
            for e in range(NBF):
                encU_h[g, :, e * W + off : e * W + off + n] = bT[:, NF8 + e, :]
            if NF8:
                blk8 = enc_f8[gb, s0 : s0 + n]
                bT8 = blk8.T.reshape(NE, 128, n).transpose(1, 0, 2)
                for k in range(NF8):
                    enc8_h[g, :, k * W + off : k * W + off + n] = bT8[:, k, :]
            dectU_h[:, :, j] = dec_bf[gb].reshape(NK, 128).T
            # scores live in BIN coordinates: unit j's valid columns are
            # [off, off + n) of scU row j
            maskl_h[j, off : off + n] = 0.0
            u2s_h[j, slot] = 1.0
            umap[j] = (gb, s0, n, off)
        im = {}
        if NF8:
            im["enc8"] = enc8_h
            im["wenc8"] = wenc8_h
        in_maps.append(
            {
                **im,
                "encU": encU_h,
                "vGT": vGT_h,
                "wenc": wenc_h,
                "wdec": wdec_h,
                "dectU": np.ascontiguousarray(dectU_h),
                "bt": bt_h,
                "maskl": maskl_h,
                "idU": idU_h,
                "u2u": u2s_h @ u2s_h.T,
            }
        )
        unit_maps.append(umap)
    return in_maps, unit_maps


def kernel(decoder_state, encoder_outputs, input_mask, W_transform, b_transform,
           v_scorer, _trace=False):
    from concourse.bass_utils import run_bass_kernel_spmd

    decoder_state = np.asarray(decoder_state)
    encoder_outputs = np.asarray(encoder_outputs)
    input_mask = np.asarray(input_mask)
    W_transform = np.asarray(W_transform)
    b_transform = np.asarray(b_transform)
    v_scorer = np.asarray(v_scorer)

    lengths = input_mask.sum(axis=1).astype(int)
    members, T = _assign(lengths)

    # NOTE: walrus --enable-ldw-opt rejects bass's pre-split InstLdweights
    # stream ("not compatible with LDW optimization"), so same-weight
    # matmul runs cannot elide reloads; DoubleRow stays weight-load-bound.
    key = ("nc", tuple(T))
    if key not in _cache:
        _cache[key] = _build(T)
    nc = _cache[key]

    in_maps, unit_maps = _prep_inputs(
        decoder_state, encoder_outputs, input_mask, W_transform, b_transform,
        v_scorer, members, T
    )
    res = run_bass_kernel_spmd(nc, in_maps, core_ids=list(range(N_CORES)), trace=_trace)

    out_full = np.zeros((B, S), dtype=np.float32)
    for core in range(N_CORES):
        o = res.results[core]["out"]  # [U, MAXW]
        for j, um in enumerate(unit_maps[core]):
            if um is None:
                continue
            gb, s0, n, off = um
            out_full[gb, s0 : s0 + n] = o[j, off : off + n]
    if _trace:
        _cache["last_result"] = res
    return out_full


# revision 73
# speedup vs baseline: 1.1778x; 1.1778x over previous
"""Bahdanau-style attention kernel for Trainium2, SPMD across 8 NeuronCores.

Math (per batch row b):
    dec_proj = decoder_state @ W_dec + b_transform            # [D]
    enc_proj = encoder_outputs[b] @ W_enc                     # [S, D]
    feats    = tanh(enc_proj + dec_proj)                      # [S, D]
    scores   = feats @ v_scorer                               # [S]
    probs    = softmax(where(mask, scores, -1e9))             # [S]

Distribution: data-parallel on batch (weights replicated) with mask-aware
work packing. The mask is length-style, so positions >= length contribute
exactly 0. Work is split into variable-width units (one contiguous chunk
of one batch, width <= 512); a COMMON width template T[0..U-1] is chosen
so every core's sorted chunk profile fits under it (SPMD: all cores run
the same graph). Template slots are first-fit packed into "bins" of
<= 512 columns; each bin is one PSUM bank and one matmul per (t, e)
weight tile, so the 512-column moving operand amortizes LDWEIGHTS and
per-instruction overheads.

Device-side formulation (per core):
  - Transposed layout enc_projT[d, s]: lhsT (stationary) = W_enc tiles in
    natural [e, d] layout, moving operand = encT[e, s] (host pre-packs
    bins contiguously per partition + casts bf16), dec_proj bias rides
    the partition axis (native ScalarE activation bias), v-dot is a
    K=D matmul per unit.
  - Unit u's scores land on partition u of one PSUM tile [U, 512] via
    one-hot-column v selectors; the additive log-mask matmul OPENS the
    accumulation group covering the full [U, 512] (variable-width score
    matmuls only cover prefixes), score matmuls accumulate behind it.
  - Softmax without max-subtraction (scores are tanh-bounded):
    probs = exp(s + logmask) / sum; per-batch sums are formed from
    per-unit row sums with two tiny matmuls against 0/1 unit<->slot
    maps, so no cross-partition vector ops are needed.
"""

import math
import random

import numpy as np
import ml_dtypes

# NOTE: walrus's --enable-ldw-opt pass (which could elide redundant
# LDWEIGHTS for same-weight matmul runs) rejects bass's pre-split
# InstLdweights stream, so DoubleRow matmuls stay weight-load-bound
# (~310ns vs ~146ns stream time) — no way to amortize reloads from here.

B, S, E, D = 64, 1024, 1024, 512
N_CORES = 8
BPC = B // N_CORES  # batch slots per core
NE = E // 128  # 8 e-tiles
ND = D // 128  # 4 d-tiles
NK = D // 128  # 4 k-tiles for W_dec (K = DEC_DIM = 512)
MAXW = 512  # PSUM bank width (512 f32) — scU/out shapes
# max unit/bin width: matmuls at exactly N=512 (full PSUM bank) measure
# ~+54ns slower than N<=508, and fp8 DoubleRow subtile steps must be
# 16-byte aligned, so widths are 16-aligned and capped at 496
CAPW = 496
ALIGN = 16
# e-tile pairs computed in fp8e4m3 via DoubleRow (out of NE//2 = 4 pairs);
# remaining e-tiles stay bf16.  2 pairs (half the contraction) measures
# rel_err ~1.5e-2 in simulation vs the 2e-2 gate; 0 = all-bf16 fallback.
FP8_PAIRS = 2
WSCALE = 32.0  # pow-2 prescale on W_enc so fp8 weights avoid denormals
NEG = -1e9

_cache = {}


# ---------------------------------------------------------------------------
# packing: choose per-core batch assignment + common width template
# ---------------------------------------------------------------------------

def _chunks(l):
    """Split length l into ceil(l/CAPW) near-equal pieces (desc order)."""
    k = max(1, math.ceil(l / CAPW))
    base, rem = divmod(l, k)
    return sorted(
        (base + (1 if i < rem else 0) for i in range(k)), reverse=True
    )


def _profile(core_batches, lengths, U):
    ws = []
    for b in core_batches:
        ws += _chunks(int(lengths[b]))
    ws.sort(reverse=True)
    if len(ws) > U:
        return None
    return ws + [0] * (U - len(ws))


def _template_cost(members, lengths, U):
    profs = [_profile(m, lengths, U) for m in members]
    if any(p is None for p in profs):
        return None, None
    T = [
        min(
            CAPW,
            (max(p[j] for p in profs) + ALIGN - 1) // ALIGN * ALIGN,
        )
        for j in range(U)
    ]
    T = [t for t in T if t > 0]
    return sum(T), T


def _assign(lengths):
    """Assign batches to cores + build the common width template.

    Returns (members, T): members[i] = batch ids on core i (slot order),
    T = template widths (desc, 4-aligned, <= MAXW).
    """
    order = sorted(range(B), key=lambda b: -int(lengths[b]))
    loads = [0] * N_CORES
    members = [[] for _ in range(N_CORES)]
    for b in order:
        cands = [i for i in range(N_CORES) if len(members[i]) < BPC]
        i = min(cands, key=lambda i: (loads[i], len(members[i])))
        members[i].append(b)
        loads[i] += int(lengths[b])

    U = max(
        sum(math.ceil(max(1, int(lengths[b])) / MAXW) for b in m)
        for m in members
    ) + 1
    rng = random.Random(1234)
    bestc, bestT = _template_cost(members, lengths, U)
    bestm = [list(m) for m in members]
    cur = [list(m) for m in members]
    curc = bestc
    for _ in range(30000):
        i = rng.randrange(N_CORES)
        j = rng.randrange(N_CORES)
        if i == j:
            continue
        a = rng.randrange(len(cur[i]))
        b2 = rng.randrange(len(cur[j]))
        cur[i][a], cur[j][b2] = cur[j][b2], cur[i][a]
        c, T = _template_cost(cur, lengths, U)
        if c is not None and c <= curc:
            curc = c
            if c < bestc:
                bestc, bestT = c, T
                bestm = [list(m) for m in cur]
        else:
            cur[i][a], cur[j][b2] = cur[j][b2], cur[i][a]
    return bestm, bestT


def _bins_of(T):
    """First-fit-decreasing template slots into bins of <= CAPW columns.

    Returns list of bins; each bin is a list of slot indices.
    """
    bins = []
    widths = []
    for j in range(len(T)):  # T already sorted desc
        for bi, w in enumerate(widths):
            if w + T[j] <= CAPW:
                bins[bi].append(j)
                widths[bi] += T[j]
                break
        else:
            bins.append([j])
            widths.append(T[j])
    # NOTE: sorting multi-slot bins first (to shrink the tail's last-ACT
    # wait) measured ~1us WORSE — heavier early ACT load during the
    # DMA-tight startup outweighs the tail gain; keep FFD order
    return bins


# ---------------------------------------------------------------------------
# device graph
# ---------------------------------------------------------------------------

def _build(T):
    """Build + compile the SPMD graph for width template T."""
    from contextlib import ExitStack

    import concourse.bass as bass
    import concourse.tile as tile
    from concourse import bacc, mybir

    f32 = mybir.dt.float32
    bf16 = mybir.dt.bfloat16
    f8 = mybir.dt.float8e4
    AF = mybir.ActivationFunctionType
    DR = mybir.MatmulPerfMode.DoubleRow

    U = len(T)
    bins = _bins_of(T)
    NG = len(bins)
    binw = [sum(T[j] for j in bb) for bb in bins]  # bin total width
    # per-slot offset within its bin
    sloff = {}
    for bb in bins:
        off = 0
        for j in bb:
            sloff[j] = off
            off += T[j]
    NF8 = 2 * FP8_PAIRS  # e-tiles in fp8
    NBF = NE - NF8  # e-tiles in bf16
    F8W = NF8 * MAXW  # per-partition flat capacity of a bin's fp8 blob
    FBW = NBF * MAXW  # ... and its bf16 blob

    nc = bacc.Bacc(
        "TRN2", target_bir_lowering=False, debug=False, num_devices=N_CORES
    )

    if NF8:
        enc8 = nc.dram_tensor("enc8", [NG, 128, F8W], f8, kind="ExternalInput").ap()
        wenc8 = nc.dram_tensor(
            "wenc8", [128, ND, FP8_PAIRS, 2, 128], f8, kind="ExternalInput"
        ).ap()
    encU = nc.dram_tensor("encU", [NG, 128, FBW], bf16, kind="ExternalInput").ap()
    wenc = nc.dram_tensor("wenc", [ND, 128, NBF, 128], bf16, kind="ExternalInput").ap()
    wdec = nc.dram_tensor("wdec", [128, NK, D], bf16, kind="ExternalInput").ap()
    dectU = nc.dram_tensor("dectU", [128, NK, U], bf16, kind="ExternalInput").ap()
    bt = nc.dram_tensor("bt", [128, ND], f32, kind="ExternalInput").ap()
    # multi-hot v selector: vGT[:, g, t, j] = v tile t if slot j is in bin g
    # else 0 — ONE score matmul per (bin, t) scores every unit of the bin
    # simultaneously; stray products land on masked columns of other rows
    vGT = nc.dram_tensor("vGT", [128, NG, ND, U], bf16, kind="ExternalInput").ap()
    # additive log-mask (0 valid / -1e9 masked) + identity; its matmul OPENS
    # the score accumulation group (start=True) covering the full [U, MAXW]
    maskl = nc.dram_tensor("maskl", [U, MAXW], bf16, kind="ExternalInput").ap()
    idU = nc.dram_tensor("idU", [U, U], bf16, kind="ExternalInput").ap()
    # unit<->unit same-batch indicator: M[u', u] = 1 iff batch(u') == batch(u);
    # one matmul turns per-unit row sums into per-unit BATCH sums directly
    u2u = nc.dram_tensor("u2u", [U, U], f32, kind="ExternalInput").ap()
    out = nc.dram_tensor("out", [U, MAXW], f32, kind="ExternalOutput").ap()

    with tile.TileContext(nc) as tc:
        with ExitStack() as ctx:
            const = ctx.enter_context(tc.tile_pool(name="const", bufs=1))
            enc_pool = ctx.enter_context(tc.tile_pool(name="encp", bufs=6))
            fpool = ctx.enter_context(tc.tile_pool(name="feats", bufs=12))

            # PE warmup: the memset is the very first gpsimd instruction so
            # the dependency-free dummy matmuls can start ASAP, fill the
            # startup DMA wait, and trip the HAM clock-gate up.  Sized to
            # END as bin 0's data lands (~2us): 12 N=512 warmups measured
            # ~5.2us at cold clock and DELAYED the first real matmul ~3.3us
            warm_sb = const.tile([128, 384], bf16)
            nc.gpsimd.memset(warm_sb[:], 0.0)

            # HBM bandwidth is the startup constraint, so everything rides
            # ONE ring in dependency order: dec-phase weights, then the t=0
            # d-tile of W_enc + bin 0 (split per-e so the first matmul only
            # waits for its own slice), then the rest
            wdec_sb = const.tile([128, NK, D], bf16)
            nc.sync.dma_start(wdec_sb[:], wdec)
            dectU_sb = const.tile([128, NK, U], bf16)
            nc.sync.dma_start(dectU_sb[:], dectU)
            # tiny, but consumed by the score-group opener matmul which sits
            # early in the PE program order — load before the big bin DMAs
            maskl_sb = const.tile([U, MAXW], bf16)
            nc.sync.dma_start(maskl_sb[:], maskl)
            idU_sb = const.tile([U, U], bf16)
            nc.sync.dma_start(idU_sb[:], idU)
            wenc_sb = const.tile([128, ND, NBF, 128], bf16)
            if NF8:
                w8_sb = const.tile([128, ND, FP8_PAIRS, 2, 128], f8)
                nc.sync.dma_start(w8_sb[:], wenc8)

            def load_bin(g, split=False):
                W = binw[g]
                et8 = None
                if NF8:
                    et8 = enc_pool.tile([128, F8W], f8, tag="et8", name="et8")
                    if split:
                        for k in range(NF8):
                            nc.sync.dma_start(
                                et8[:, k * W : (k + 1) * W],
                                enc8[g, :, k * W : (k + 1) * W],
                            )
                    else:
                        nc.sync.dma_start(
                            et8[:, 0 : NF8 * W], enc8[g, :, 0 : NF8 * W]
                        )
                etb = enc_pool.tile([128, FBW], bf16, tag="etb", name="etb")
                if split:
                    for e in range(NBF):
                        nc.sync.dma_start(
                            etb[:, e * W : (e + 1) * W],
                            encU[g, :, e * W : (e + 1) * W],
                        )
                else:
                    nc.sync.dma_start(etb[:, 0 : NBF * W], encU[g, :, 0 : NBF * W])
                return et8, etb

            ets = []
            # prefetch 3 bins ahead of the late constants (vGT etc.) on the
            # ring — bin 2's data otherwise queues behind them, and the
            # early-run PE gaps sit exactly at the bin 1/2 boundaries
            npre = min(3, NG)
            nc.sync.dma_start(wenc_sb[:, 0, :, :], wenc[0])
            ets.append(load_bin(0, split=True))
            for t in range(1, ND):
                nc.sync.dma_start(wenc_sb[:, t, :, :], wenc[t])
            for g in range(1, npre):
                # NOTE: splitting later bins per-piece (like bin 0) measures
                # 14us WORSE — 8x more, smaller DMA descriptors slow the
                # whole ring; whole-blob loads keep descriptors at ~4-6KB
                ets.append(load_bin(g))
            bt_sb = const.tile([128, ND], f32)
            nc.scalar.dma_start(bt_sb[:], bt)
            # late-needed constants go on the sync ring BEHIND the bin
            # prefetches so they don't steal HBM bandwidth at startup
            vGT_sb = const.tile([128, NG, ND, U], bf16)
            nc.sync.dma_start(vGT_sb[:], vGT)
            u2u_sb = const.tile([U, U], f32)
            nc.sync.dma_start(u2u_sb[:], u2u)
            decprojU_sb = const.tile([128, ND, U], f32)

            # 3 dummy warmups trip the clock-gate; the dec phase right after
            # is REAL work that continues warming the PE while bin 0's data
            # is still in flight, so bin 0 starts as soon as it lands
            with tc.tile_pool(name="warmp", bufs=1, space="PSUM") as wpool:
                wps = wpool.tile([128, 256], f32, name="wps")
                for _ in range(3):
                    nc.tensor.matmul(
                        wps[:],
                        lhsT=warm_sb[:, 0:128],
                        rhs=warm_sb[:, 128:384],
                        start=True,
                        stop=True,
                        skip_group_check=True,
                    )

            # --- dec_projT[d, u] = W_dec^T @ dec_colsU + b ---
            with tc.tile_pool(name="dpsum", bufs=2, space="PSUM") as dpsum:
                for t in range(ND):
                    ps = dpsum.tile([128, U], f32, name="dps")
                    for k in range(NK):
                        nc.tensor.matmul(
                            ps[:],
                            lhsT=wdec_sb[:, k, bass.ts(t, 128)],
                            rhs=dectU_sb[:, k, :],
                            start=(k == 0),
                            stop=(k == NK - 1),
                        )
                    nc.scalar.add(decprojU_sb[:, t, :], ps[:], bt_sb[:, t : t + 1])

            spsum = ctx.enter_context(tc.tile_pool(name="spsum", bufs=1, space="PSUM"))
            scU = spsum.tile([U, MAXW], f32, name="scU")
            n_sc_mms = NG * ND + 1  # +1 for the log-mask opener
            state = {"count": 0}
            pending = []  # delayed score MMs: (u, t, ft)

            # the log-mask matmul opens the accumulation group, covering the
            # full [U, MAXW] so the variable-width score matmuls (which only
            # write prefixes) always accumulate onto initialized PSUM
            state["count"] += 1
            nc.tensor.matmul(
                scU[:],
                lhsT=idU_sb[:],
                rhs=maskl_sb[:],
                start=True,
                stop=False,
                skip_group_check=True,
            )

            def flush_pending():
                for (g2, t, ft) in pending:
                    w = binw[g2]
                    state["count"] += 1
                    nc.tensor.matmul(
                        scU[:, 0:w],
                        lhsT=vGT_sb[:, g2, t, :],
                        rhs=ft[:, 0:w],
                        start=False,
                        stop=(state["count"] == n_sc_mms),
                        skip_group_check=True,
                    )
                pending.clear()

            # --- main loop over bins ---
            # 7 main-loop banks + the score bank = all 8 PSUM banks; the
            # extra sweep in flight buys slack against tanh-ACT drain
            with tc.tile_pool(name="mpsum", bufs=7, space="PSUM") as mpsum:
                for g in range(NG):
                    W = binw[g]
                    if g < npre:
                        et8, etb = ets[g]
                    else:
                        et8, etb = load_bin(g)
                    prev = list(pending)
                    pending.clear()
                    this_bin = []
                    for t in range(ND):
                        ps = mpsum.tile([128, MAXW], f32, tag="mp", name="mp")
                        for p in range(FP8_PAIRS):
                            # DoubleRow: lhsT [128, 2, 128] / rhs [128, 2, W]
                            # contract BOTH e-subtiles of the pair in one MM
                            nc.tensor.matmul(
                                ps[:, 0:W],
                                lhsT=w8_sb[:, t, p, :, :],
                                rhs=et8[:, 2 * p * W : (2 * p + 2) * W].rearrange(
                                    "a (k w) -> a k w", k=2
                                ),
                                start=(p == 0),
                                stop=False,
                                perf_mode=DR,
                            )
                        for e in range(NBF):
                            nc.tensor.matmul(
                                ps[:, 0:W],
                                lhsT=wenc_sb[:, t, e, :],
                                rhs=etb[:, e * W : e * W + W],
                                start=(NF8 == 0 and e == 0),
                                stop=(e == NBF - 1),
                            )
                        if t == 0 and prev:
                            # emit previous bin's score MMs behind the first
                            # weight sweep (their tanh inputs are ready, so
                            # PE doesn't stall on ACT)
                            pending.extend(prev)
                            prev = []
                            flush_pending()
                        # one shared feature tile per (bin, t); ACTs write
                        # disjoint slices, the multi-hot score MM reads it all
                        ft = fpool.tile([128, MAXW], bf16, tag="ft", name="ft")
                        for j in bins[g]:
                            off = sloff[j]
                            nc.scalar.activation(
                                ft[:, off : off + T[j]],
                                ps[:, off : off + T[j]],
                                func=AF.Tanh,
                                bias=decprojU_sb[:, t, j : j + 1],
                                scale=1.0 / WSCALE,
                            )
                        this_bin.append((g, t, ft))
                    pending.extend(this_bin)
                flush_pending()

            # --- masked softmax epilogue in unit space ---
            with tc.tile_pool(name="tpsum", bufs=1, space="PSUM") as tpsum, \
                 tc.tile_pool(name="epi", bufs=1) as epool:
                escU = epool.tile([U, MAXW], f32, name="escU")
                usums = epool.tile([U, 1], f32, name="usums")
                nc.scalar.activation(
                    escU[:], scU[:], func=AF.Exp, accum_out=usums[:]
                )
                # per-unit BATCH sums in one matmul: bU[u] = sum_u' M[u',u]*usums[u']
                bU_ps = tpsum.tile([U, 1], f32, name="bU_ps")
                nc.tensor.matmul(
                    bU_ps[:], lhsT=u2u_sb[:], rhs=usums[:], start=True, stop=True
                )
                rbU = epool.tile([U, 1], f32, name="rbU")
                nc.vector.reciprocal(rbU[:], bU_ps[:])
                # split the final scale + store into column halves on two
                # DMA rings so the first store overlaps the second scale
                probsU = epool.tile([U, MAXW], f32, name="probsU")
                H = MAXW // 2
                nc.vector.tensor_scalar_mul(probsU[:, 0:H], escU[:, 0:H], rbU[:])
                nc.sync.dma_start(out[:, 0:H], probsU[:, 0:H])
                nc.vector.tensor_scalar_mul(probsU[:, H:MAXW], escU[:, H:MAXW], rbU[:])
                nc.scalar.dma_start(out[:, H:MAXW], probsU[:, H:MAXW])

    nc.compile()
    return nc


# ---------------------------------------------------------------------------
# host-side data prep
# ---------------------------------------------------------------------------

def _prep_inputs(decoder_state, encoder_outputs, input_mask, W_transform,
                 b_transform, v_scorer, members, T):
    bf16 = ml_dtypes.bfloat16
    f8 = ml_dtypes.float8_e4m3
    W_dec = W_transform[:D]
    W_enc = W_transform[D:]
    U = len(T)
    bins = _bins_of(T)
    NG = len(bins)
    binw = [sum(T[j] for j in bb) for bb in bins]
    sloff = {}
    for bb in bins:
        off = 0
        for j in bb:
            sloff[j] = off
            off += T[j]
    NF8 = 2 * FP8_PAIRS
    NBF = NE - NF8
    F8W = NF8 * MAXW
    FBW = NBF * MAXW

    W32 = W_enc * np.float32(WSCALE)
    # bf16 e-tiles NF8..NE-1: [e, d] -> [t_d, p, t_e, dcol]
    wenc_h = np.ascontiguousarray(
        W32.astype(bf16).reshape(NE, 128, ND, 128).transpose(2, 1, 0, 3)[
            :, :, NF8:, :
        ]
    )
    if NF8:
        # fp8 e-tiles as DoubleRow pairs: [128, ND, pair, sub, dcol]
        wenc8_h = np.ascontiguousarray(
            W32[: NF8 * 128]
            .astype(f8)
            .reshape(FP8_PAIRS, 2, 128, ND, 128)
            .transpose(2, 3, 0, 1, 4)
        )
    wdec_h = np.ascontiguousarray(
        W_dec.astype(bf16).reshape(NK, 128, D).transpose(1, 0, 2)
    )
    bt_h = np.ascontiguousarray(b_transform.astype(np.float32).reshape(ND, 128).T)
    v_tiles = v_scorer.astype(np.float32).reshape(ND, 128).T.astype(bf16)  # [128, ND]

    lengths = input_mask.sum(axis=1).astype(int)
    enc_bf = encoder_outputs.astype(bf16)  # [B, S, E]
    enc_f8 = encoder_outputs.astype(f8) if NF8 else None
    dec_bf = decoder_state.astype(bf16)  # [B, D]

    in_maps = []
    unit_maps = []  # per core: list of (global_batch, s0, n) per slot (None = dummy)
    for core in range(N_CORES):
        batches = members[core]
        # per-core units: even chunks, sorted desc — matches template order
        units = []  # (width, gb, slot_in_core, s0)
        for slot, gb in enumerate(batches):
            l = int(lengths[gb])
            s0 = 0
            for wdt in _chunks(l):
                units.append((wdt, gb, slot, s0))
                s0 += wdt
        units.sort(key=lambda x: -x[0])
        assert len(units) <= U
        assert all(u[0] <= T[j] for j, u in enumerate(units))

        encU_h = np.zeros((NG, 128, FBW), dtype=bf16)
        enc8_h = np.zeros((NG, 128, F8W), dtype=f8) if NF8 else None
        dectU_h = np.zeros((128, NK, U), dtype=bf16)
        vGT_h = np.zeros((128, NG, ND, U), dtype=bf16)
        for bi, bb in enumerate(bins):
            for j in bb:
                for t in range(ND):
                    vGT_h[:, bi, t, j] = v_tiles[:, t]
        maskl_h = np.full((U, MAXW), NEG, dtype=bf16)
        idU_h = np.eye(U, dtype=bf16)
        u2s_h = np.zeros((U, BPC), dtype=np.float32)
        umap = [None] * U
        for j, (n, gb, slot, s0) in enumerate(units):
            g = next(bi for bi, bb in enumerate(bins) if j in bb)
            W = binw[g]
            off = sloff[j]
            # [n, E] -> [E, n] -> [NE, 128, n] -> [128, NE, n]
            blk = enc_bf[gb, s0 : s0 + n]
            bT = blk.T.reshape(NE, 128, n).transpose(1, 0, 2)
            for e in range(NBF):
                encU_h[g, :, e * W + off : e * W + off + n] = bT[:, NF8 + e, :]
            if NF8:
                blk8 = enc_f8[gb, s0 : s0 + n]
                bT8 = blk8.T.reshape(NE, 128, n).transpose(1, 0, 2)
                for k in range(NF8):
                    enc8_h[g, :, k * W + off : k * W + off + n] = bT8[:, k, :]
            dectU_h[:, :, j] = dec_bf[gb].reshape(NK, 128).T
            # scores live in BIN coordinates: unit j's valid columns are
            # [off, off + n) of scU row j
            maskl_h[j, off : off + n] = 0.0
            u2s_h[j, slot] = 1.0
            umap[j] = (gb, s0, n, off)
        im = {}
        if NF8:
            im["enc8"] = enc8_h
            im["wenc8"] = wenc8_h
        in_maps.append(
            {
                **im,
                "encU": encU_h,
                "vGT": vGT_h,
                "wenc": wenc_h,
                "wdec": wdec_h,
                "dectU": np.ascontiguousarray(dectU_h),
                "bt": bt_h,
                "maskl": maskl_h,
                "idU": idU_h,
                "u2u": u2s_h @ u2s_h.T,
            }
        )
        unit_maps.append(umap)
    return in_maps, unit_maps


def kernel(decoder_state, encoder_outputs, input_mask, W_transform, b_transform,
           v_scorer, _trace=False):
    from concourse.bass_utils import run_bass_kernel_spmd

    decoder_state = np.asarray(decoder_state)
    encoder_outputs = np.asarray(encoder_outputs)
    input_mask = np.asarray(input_mask)
    W_transform = np.asarray(W_transform)
    b_transform = np.asarray(b_transform)
    v_scorer = np.asarray(v_scorer)

    lengths = input_mask.sum(axis=1).astype(int)
    members, T = _assign(lengths)

    # NOTE: walrus --enable-ldw-opt rejects bass's pre-split InstLdweights
    # stream ("not compatible with LDW optimization"), so same-weight
    # matmul runs cannot elide reloads; DoubleRow stays weight-load-bound.
    key = ("nc", tuple(T))
    if key not in _cache:
        _cache[key] = _build(T)
    nc = _cache[key]

    in_maps, unit_maps = _prep_inputs(
        decoder_state, encoder_outputs, input_mask, W_transform, b_transform,
        v_scorer, members, T
    )
    res = run_bass_kernel_spmd(nc, in_maps, core_ids=list(range(N_CORES)), trace=_trace)

    out_full = np.zeros((B, S), dtype=np.float32)
    for core in range(N_CORES):
        o = res.results[core]["out"]  # [U, MAXW]
        for j, um in enumerate(unit_maps[core]):
            if um is None:
                continue
            gb, s0, n, off = um
            out_full[gb, s0 : s0 + n] = o[j, off : off + n]
    if _trace:
        _cache["last_result"] = res
    return out_full
